# revision 4
# baseline (speedup 1.0000x reference)
"""Exponential Hawkes process negative log-likelihood on 8 Trainium2 cores.

Math (reference):
    R_0 = 0;  R_i = exp(-beta*(t_i - t_{i-1})) * (1 + R_{i-1})
    lam_i = mu + alpha * R_i
    nll = -[ sum_i log(lam_i) - mu*T - (alpha/beta) * sum_i (1 - exp(-beta*(T - t_i)))
             - 1000 * relu(alpha/beta - 0.999)^2 ]

Strategy:
  - Shard the 8.4M event axis across 8 cores, each shard prefixed with an
    8192-event halo so the incoming recurrence carry is reproduced locally
    (exp(-beta * halo_span) underflows to 0 in f32, so this is exact).
    Core 0 is front-padded with events 1e6 time units in the past, which
    forces its carry to exactly 0.
  - Per core the (halo+shard) sequence is laid out [128, C]: partition p
    owns a contiguous chunk of C events.  Per tile of F columns:
      dt   = t - t_prev                          (DVE shifted subtract)
      a    = exp(-beta*dt)                       (ACT)
      B    = scan: B_c = a_c*(1+B_{c-1})         (DVE tensor_tensor_scan,
                                                  chained across tiles)
      Ap   = exp(-beta*(t - chunk_prev[p]))      (ACT, per-partition bias)
    Then a 128-element cross-partition affine carry scan (column -> row via
    SBUF DMA, scan on one partition, shift, row -> column), and
      R    = B + Ap * K[p]                       (DVE scalar_tensor_tensor)
      logl = Ln(alpha*R + mu), accumulated       (ACT, accum_out)
    Ap underflow to 0 deep inside a chunk is benign: the true carry
    contribution there is below f32 resolution anyway.
  - The integral's exp(-beta*(T - t_i)) is only nonzero (in f32) for events
    within ~104/beta of T; the pass runs on enough trailing column-tiles to
    cover those (computed from the data at build time), using t - T formed
    in f32 BEFORE scaling by beta to keep the small difference exact.
  - Each core returns per-(partition, tile) partial sums; the host masks the
    halo entries and reduces in f64.
"""

import numpy as np

# Problem constants (hardcoded per task instructions).
N = 8_388_608          # total events
M = 8                  # cores
S = N // M             # events per shard (1,048,576)
H = 8192               # halo events prepended to each shard
L = S + H              # per-core sequence length
P = 128                # SBUF partitions
C = L // P             # columns per partition (8256)
F = 512                # column-tile width
EPS = 1e-8
PENALTY = 1000.0
PAD_GAP = 1.0e6        # core-0 pad offset; exp(-beta*PAD_GAP) == 0 in f32

# Column tiles: (start, width)
_TILES = [(j * F, F) for j in range(C // F)]
if C % F:
    _TILES.append(((C // F) * F, C % F))
NT = len(_TILES)

_PROGRAM_CACHE: dict = {}


def _softplus64(x: float) -> float:
    return float(np.logaddexp(0.0, np.float64(x)))


def _build_program(beta: float, mu: float, alpha: float, T: float,
                   n_int_tiles: int):
    import concourse.bacc as bacc
    import concourse.mybir as mybir
    from concourse.tile import TileContext

    f32 = mybir.dt.float32
    AF = mybir.ActivationFunctionType
    OP = mybir.AluOpType

    nc = bacc.Bacc()
    ev = nc.dram_tensor("ev", [P, C], f32, kind="ExternalInput")
    prev = nc.dram_tensor("prev", [P, 1], f32, kind="ExternalInput")
    out_log = nc.dram_tensor("out_log", [P, NT], f32, kind="ExternalOutput")
    out_int = nc.dram_tensor("out_int", [P, NT], f32, kind="ExternalOutput")

    with TileContext(nc) as tc:
        with tc.tile_pool(name="pers", bufs=1) as pers, \
             tc.tile_pool(name="work", bufs=3) as work:
            Bfull = pers.tile([P, C], f32)
            Apfull = pers.tile([P, C], f32)
            logstat = pers.tile([P, NT], f32)
            intstat = pers.tile([P, NT], f32)
            prevsb = pers.tile([P, 1], f32)
            bprev = pers.tile([P, 1], f32)
            ksb = pers.tile([P, 1], f32)
            musb = pers.tile([P, 1], f32)
            rowAB = pers.tile([1, 2 * P], f32)
            rowR = pers.tile([1, P], f32)
            rowK = pers.tile([1, P], f32)

            nc.gpsimd.memset(logstat[:], 0.0)
            nc.gpsimd.memset(intstat[:], 0.0)
            nc.gpsimd.memset(musb[:], float(mu))
            nc.sync.dma_start(prevsb[:], prev[:])
            # per-partition bias for the Ap pass: +beta * prev[p]
            nc.vector.tensor_scalar(bprev[:], prevsb[:], float(beta), None,
                                    OP.mult)

            # ---- Phase 1: dt -> a -> local scan B; Ap; (trailing) int exp
            for j, (c0, w) in enumerate(_TILES):
                ext = work.tile([P, F + 1], f32, tag="ext")
                if j == 0:
                    nc.vector.tensor_copy(ext[:, 0:1], prevsb[:])
                    nc.sync.dma_start(ext[:, 1:1 + w], ev[:, 0:w])
                else:
                    nc.sync.dma_start(ext[:, 0:w + 1], ev[:, c0 - 1:c0 + w])

                dtt = work.tile([P, F], f32, tag="dtt")
                nc.vector.tensor_tensor(dtt[:, :w], ext[:, 1:w + 1],
                                        ext[:, 0:w], OP.subtract)
                at = work.tile([P, F], f32, tag="at")
                nc.scalar.activation(at[:, :w], dtt[:, :w], AF.Exp,
                                     scale=float(-beta))
                init = 0.0 if j == 0 else Bfull[:, c0 - 1:c0]
                nc.vector.tensor_tensor_scan(
                    Bfull[:, c0:c0 + w], at[:, :w], at[:, :w], init,
                    op0=OP.mult, op1=OP.add)
                nc.scalar.activation(Apfull[:, c0:c0 + w], ext[:, 1:w + 1],
                                     AF.Exp, scale=float(-beta),
                                     bias=bprev[:])
                if j >= NT - n_int_tiles:
                    # (t - T) in f32 first (exact near T), then *beta in ACT
                    dtT = work.tile([P, F], f32, tag="dtT")
                    nc.vector.tensor_scalar(dtT[:, :w], ext[:, 1:w + 1],
                                            float(-T), None, OP.add)
                    eint = work.tile([P, F], f32, tag="eint")
                    nc.scalar.activation(eint[:, :w], dtT[:, :w], AF.Exp,
                                         scale=float(beta),
                                         accum_out=intstat[:, j:j + 1])

            # ---- Cross-partition carry: K[p] = R at end of partition p-1
            # column -> row via SBUF->SBUF DMA (works, probed)
            nc.sync.dma_start(rowAB[0:1, 0:P], Apfull[:, C - 1:C])
            nc.sync.dma_start(rowAB[0:1, P:2 * P], Bfull[:, C - 1:C])
            nc.vector.tensor_tensor_scan(
                rowR[0:1, :], rowAB[0:1, 0:P], rowAB[0:1, P:2 * P], 0.0,
                op0=OP.mult, op1=OP.add)
            nc.gpsimd.memset(rowK[0:1, 0:1], 0.0)
            nc.vector.tensor_copy(rowK[0:1, 1:P], rowR[0:1, 0:P - 1])
            nc.sync.dma_start(ksb[:, :], rowK[0:1, 0:P])

            # ---- Phase 2: R = B + Ap*K; log-lik accumulate
            for j, (c0, w) in enumerate(_TILES):
                rbuf = work.tile([P, F], f32, tag="rbuf")
                nc.vector.scalar_tensor_tensor(
                    rbuf[:, :w], Apfull[:, c0:c0 + w], ksb[:],
                    Bfull[:, c0:c0 + w], op0=OP.mult, op1=OP.add)
                lnl = work.tile([P, F], f32, tag="lnl")
                nc.scalar.activation(lnl[:, :w], rbuf[:, :w], AF.Ln,
                                     scale=float(alpha), bias=musb[:],
                                     accum_out=logstat[:, j:j + 1])

            nc.sync.dma_start(out_log[:], logstat[:])
            nc.sync.dma_start(out_int[:], intstat[:])

    nc.finalize()
    return nc


def _get_program(beta, mu, alpha, T, n_int_tiles):
    key = (repr(beta), repr(mu), repr(alpha), repr(T), n_int_tiles)
    prog = _PROGRAM_CACHE.get(key)
    if prog is None:
        prog = _build_program(beta, mu, alpha, T, n_int_tiles)
        _PROGRAM_CACHE[key] = prog
    return prog


def kernel(event_times, raw_mu, raw_alpha, raw_beta, _want_trace=False):
    from concourse.bass_utils import run_bass_kernel_spmd

    ev_full = np.ascontiguousarray(np.asarray(event_times, dtype=np.float32))
    assert ev_full.shape == (N,), ev_full.shape
    mu = _softplus64(float(np.asarray(raw_mu))) + EPS
    alpha = _softplus64(float(np.asarray(raw_alpha))) + EPS
    beta = _softplus64(float(np.asarray(raw_beta))) + EPS
    T = float(ev_full[-1])

    # Trailing tiles needed so every event with beta*(T - t) <= ~104 (the
    # f32 exp underflow point) is covered by the integral pass; 4x margin.
    cnt = int(N - np.searchsorted(ev_full, np.float32(T - 130.0 / beta)))
    cover = max(256, 4 * cnt)
    n_int_tiles, acc = 0, 0
    for c0, w in reversed(_TILES):
        if acc >= cover:
            break
        n_int_tiles += 1
        acc += w
    prog = _get_program(beta, mu, alpha, T, n_int_tiles)

    # Per-core inputs: halo+shard window and per-partition-chunk predecessors
    in_maps = []
    for k in range(M):
        if k == 0:
            win = np.empty(L, np.float32)
            win[:H] = ev_full[0] - np.float32(PAD_GAP)
            win[H:] = ev_full[:S]
            prev0 = ev_full[0] - np.float32(2 * PAD_GAP)
        else:
            win = ev_full[k * S - H:(k + 1) * S]
            prev0 = ev_full[k * S - H - 1]
        pv = np.empty(P, np.float32)
        pv[0] = prev0
        pv[1:] = win[C - 1:L - 1:C]
        in_maps.append({"ev": win.reshape(P, C), "prev": pv.reshape(P, 1)})

    res = run_bass_kernel_spmd(prog, in_maps, list(range(M)),
                               trace=_want_trace)

    log_term = np.float64(0.0)
    int_sum = np.float64(0.0)
    for k in range(M):
        lg = res.results[k]["out_log"].astype(np.float64)
        ii = res.results[k]["out_int"].astype(np.float64)
        for j, (c0, w) in enumerate(_TILES):
            if c0 + w <= H:          # partition-0 columns of this tile = halo
                lg[0, j] = 0.0
                ii[0, j] = 0.0
        log_term += lg.sum()
        int_sum += ii.sum()

    integral_term = mu * T + (alpha / beta) * (N - int_sum)
    branching = alpha / beta
    penalty = PENALTY * max(branching - 0.999, 0.0) ** 2
    loglik = log_term - integral_term - penalty
    out = np.float32(-loglik)
    if _want_trace:
        return out, res
    return out
